# revision 1
# baseline (speedup 1.0000x reference)
"""Trainium2 (Bass/Tile) kernel for the BiGNN layer problem.

Self-contained: hardcodes N=50000, D=256, V=2, 8 NeuronCores.
Entry point: kernel(**inputs) -> np.ndarray [50000, 256] float32.

Math: for each of 4 edge sets s (bw_v0, bw_v1, fw_v0, fw_v1):
    h_s = relu( segsum_dst(x[src] @ W_s) / max(deg_dst,1) + b_s )   [N, 128]
  acc = [h_bw0 + h_bw1 | h_fw0 + h_fw1]                            [N, 256]
  out = acc @ W1 + b1 + x      (relu(acc)=acc since acc >= 0)

Linearity: segsum(x[src] @ W_s) = segsum(proj_s[src]) with proj_s = x @ W_s.

Device flow per core (dst-shard of N/n_cores nodes):
  1. proj_s = x_shard @ W_s on PE -> DRAM, AllGather per set -> table_s
     [N,128] bf16 (4 collectives pipeline with phase 2 of earlier sets)
  2. per (set, dst tile): SWDGE dma_gather of proj rows for the tile's
     edges (<=2048 idx per instruction; trailing -1 pads move no data;
     per-core valid count in a Pool register); per 128-edge slot:
     one-hot O[e,d] = (dstf[e]==iota[d]) via DVE tensor_scalar (4x), then
     PE matmuls accumulate cnt[d] += O^T 1, S[d,:] += O^T G in PSUM.
  3. h = relu(S * recip(max(cnt,1)) + b_s) accumulated into persistent
     acc tile; per tile: acc^T (PE transpose), @W1, + b1 + x, DMA out.
"""

import math
from contextlib import ExitStack
from dataclasses import dataclass, field

import ml_dtypes
import numpy as np

import concourse.bacc as bacc
import concourse.bass as bass
import concourse.mybir as mybir
import concourse.tile as tile
from concourse.masks import make_identity

F32 = mybir.dt.float32
BF16 = mybir.dt.bfloat16
I16 = mybir.dt.int16
I32 = mybir.dt.int32
BF = ml_dtypes.bfloat16

T = 128   # dst rows per tile
CH = 1024  # max idxs per dma_gather instruction (HW limit)
CARVEOUT = 16384


@dataclass
class Seg:
    n_idx: int = 0        # padded length (x128, uniform across cores)
    icol0: int = 0        # column offset in idx_all
    slot0: int = 0        # first slot in this tile's G
    chunks: list = field(default_factory=list)  # of (offset, n, gcnt_index)


@dataclass
class TilePlan:
    t: int = 0
    lo: Seg = field(default_factory=Seg)
    hi: Seg = field(default_factory=Seg)
    slots: int = 0
    dcol0: int = 0        # first dstf column (== slot base) for this tile


@dataclass
class Plan:
    N: int
    NS: int
    n_cores: int
    n_tiles: int
    split: int
    IC: int = 0
    TS: int = 0
    NG: int = 0
    max_slots: int = 1
    tiles: list = field(default_factory=list)      # [set][tile] -> TilePlan
    idx_all: list = field(default_factory=list)    # per core [128, IC] int16
    dstf_all: list = field(default_factory=list)   # per core [128, TS] bf16
    gcnt_all: list = field(default_factory=list)   # per core [1, NG] int32


def make_plan(edge_sets, N, n_cores, split=32768):
    NS = N // n_cores
    assert NS * n_cores == N
    n_tiles = math.ceil(NS / T)
    S = len(edge_sets)
    plan = Plan(N=N, NS=NS, n_cores=n_cores, n_tiles=n_tiles, split=split)

    # bucket[s][c][t] = (lo_idx, lo_dmod, hi_idx, hi_dmod)
    buckets = [[[None] * n_tiles for _ in range(n_cores)] for _ in range(S)]
    for s, e in enumerate(edge_sets):
        src = np.asarray(e[0], dtype=np.int64)
        dst = np.asarray(e[1], dtype=np.int64)
        ishi = src >= split
        key = dst * 2 + ishi
        order = np.argsort(key, kind="stable")
        srcs, dsts, ishis = src[order], dst[order], ishi[order]
        keys = key[order]
        for c in range(n_cores):
            base = c * NS
            for t in range(n_tiles):
                r0, r1 = base + t * T, base + min((t + 1) * T, NS)
                i0 = np.searchsorted(keys, 2 * r0)
                i1 = np.searchsorted(keys, 2 * r1)
                seg_hi = ishis[i0:i1]
                lo_sel = ~seg_hi
                buckets[s][c][t] = (
                    srcs[i0:i1][lo_sel],
                    (dsts[i0:i1][lo_sel] - base) % T,
                    srcs[i0:i1][seg_hi] - split,
                    (dsts[i0:i1][seg_hi] - base) % T,
                )

    icol = dcol = gi = 0
    for s in range(S):
        tl = []
        for t in range(n_tiles):
            tp = TilePlan(t=t)
            nl = max(len(buckets[s][c][t][0]) for c in range(n_cores))
            nh = max(len(buckets[s][c][t][2]) for c in range(n_cores))
            nl = ((nl + T - 1) // T) * T
            nh = ((nh + T - 1) // T) * T
            tp.lo = Seg(n_idx=nl, icol0=icol, slot0=0)
            icol += nl // 16
            tp.hi = Seg(n_idx=nh, icol0=icol, slot0=nl // T)
            icol += nh // 16
            tp.slots = (nl + nh) // T
            tp.dcol0 = dcol
            dcol += tp.slots
            for seg in (tp.lo, tp.hi):
                for o in range(0, seg.n_idx, CH):
                    seg.chunks.append((o, min(CH, seg.n_idx - o), gi))
                    gi += 1
            tl.append(tp)
        plan.tiles.append(tl)
    plan.IC = max(icol, 1)
    plan.TS = max(dcol, 1)
    plan.NG = max(gi, 1)
    plan.max_slots = max((tp.slots for tl in plan.tiles for tp in tl), default=1)

    for c in range(n_cores):
        idx_all = np.full((128, plan.IC), -1, dtype=np.int16)
        dstf_all = np.full((128, plan.TS), -1.0, dtype=BF)
        gcnt = np.zeros((1, plan.NG), dtype=np.int32)
        for s in range(S):
            for tp in plan.tiles[s]:
                b = buckets[s][c][tp.t]
                for seg, which in ((tp.lo, 0), (tp.hi, 2)):
                    if seg.n_idx == 0:
                        continue
                    bidx, bdm = b[which], b[which + 1]
                    nreal = len(bidx)
                    vals = np.full(seg.n_idx, -1, dtype=np.int16)
                    vals[:nreal] = bidx.astype(np.int16)
                    dms = np.full(seg.n_idx, -1.0, dtype=np.float32)
                    dms[:nreal] = bdm
                    for (o, n, g) in seg.chunks:
                        cnt = min(max(nreal - o, 0), n)
                        if cnt == 0:
                            vals[o] = 0  # keep >=1 valid idx per chunk
                            cnt = 1
                        gcnt[0, g] = cnt
                    cols = seg.n_idx // 16
                    pat = vals.reshape(cols, 16).T
                    idx_all[:, seg.icol0:seg.icol0 + cols] = np.tile(pat, (8, 1))
                    dcols = seg.n_idx // T
                    d0 = tp.dcol0 + seg.slot0
                    dstf_all[:, d0:d0 + dcols] = dms.reshape(dcols, T).T.astype(BF)
        plan.idx_all.append(idx_all)
        plan.dstf_all.append(dstf_all)
        plan.gcnt_all.append(gcnt)
    return plan


def host_inputs(plan, c, x_full, Wall, ball, W1, b1):
    NS = plan.NS
    return {
        "x": np.ascontiguousarray(x_full[c * NS:(c + 1) * NS]).astype(np.float32),
        "Wall": Wall.astype(BF),
        "W1": W1.astype(BF),
        "ball": np.broadcast_to(ball.astype(np.float32), (128, 512)).copy(),
        "b1": np.broadcast_to(b1.astype(np.float32), (128, 256)).copy(),
        "iotab": np.broadcast_to(np.tile(np.arange(T, dtype=np.float32), plan.max_slots).astype(BF), (128, plan.max_slots * T)).copy(),
        "idx": plan.idx_all[c],
        "dstf": plan.dstf_all[c],
        "ndstf": -plan.dstf_all[c].astype(np.float32),
        "gcnt": plan.gcnt_all[c],
    }


def build_nc(plan):
    NS, n_tiles = plan.NS, plan.n_tiles
    n_cores = plan.n_cores
    D = 256
    max_slots = plan.max_slots
    max_icols = max((tp.lo.n_idx + tp.hi.n_idx) // 16 for tl in plan.tiles for tp in tl) if plan.tiles else 1

    nc = bacc.Bacc("TRN2", num_swdge_queues=4, dynamic_dma_scratch_size=CARVEOUT)
    x = nc.dram_tensor("x", [NS, D], F32, kind="ExternalInput")
    Wall = nc.dram_tensor("Wall", [D, 512], BF16, kind="ExternalInput")
    W1 = nc.dram_tensor("W1", [D, D], BF16, kind="ExternalInput")
    ball = nc.dram_tensor("ball", [128, 512], F32, kind="ExternalInput")
    b1 = nc.dram_tensor("b1", [128, D], F32, kind="ExternalInput")
    iotab = nc.dram_tensor("iotab", [128, max_slots * T], BF16, kind="ExternalInput")
    idx = nc.dram_tensor("idx", [128, plan.IC], I16, kind="ExternalInput")
    dstf = nc.dram_tensor("dstf", [128, plan.TS], BF16, kind="ExternalInput")
    ndstf = nc.dram_tensor("ndstf", [128, plan.TS], F32, kind="ExternalInput")
    gcntt = nc.dram_tensor("gcnt", [1, plan.NG], I32, kind="ExternalInput")
    y = nc.dram_tensor("y", [NS, D], F32, kind="ExternalOutput")

    proj_local = [nc.dram_tensor(f"proj_local{s}", [NS, 128], BF16) for s in range(4)]
    tables = [nc.dram_tensor(f"table{s}", [plan.N, 128], BF16) for s in range(4)]

    with tile.TileContext(nc) as tc, ExitStack() as ctx:
        const = ctx.enter_context(tc.tile_pool(name="const", bufs=1))
        sb = ctx.enter_context(tc.tile_pool(name="sb", bufs=3))
        gpool = ctx.enter_context(tc.tile_pool(name="gp", bufs=8))
        ipool = ctx.enter_context(tc.tile_pool(name="ip", bufs=8))
        opool = ctx.enter_context(tc.tile_pool(name="op", bufs=4))
        ps_s = ctx.enter_context(tc.tile_pool(name="ps_s", bufs=3, space="PSUM"))
        ps_c = ctx.enter_context(tc.tile_pool(name="ps_c", bufs=3, space="PSUM"))
        ps_t = ctx.enter_context(tc.tile_pool(name="ps_t", bufs=1, space="PSUM"))
        ps_w = ctx.enter_context(tc.tile_pool(name="ps_w", bufs=1, space="PSUM"))

        # ---- constants ----
        wall_sb = []
        w1_sb = []
        for k in range(2):
            wt = const.tile([128, 512], BF16, name=f"wall{k}")
            nc.sync.dma_start(out=wt[:], in_=Wall[k * 128:(k + 1) * 128, :])
            wall_sb.append(wt)
            w1t = const.tile([128, D], BF16, name=f"w1_{k}")
            nc.sync.dma_start(out=w1t[:], in_=W1[k * 128:(k + 1) * 128, :])
            w1_sb.append(w1t)
        ball_sb = const.tile([128, 512], F32)
        nc.sync.dma_start(out=ball_sb[:], in_=ball[:])
        b1_sb = const.tile([128, D], F32)
        nc.sync.dma_start(out=b1_sb[:], in_=b1[:])
        iota_sb = const.tile([128, max_slots * T], BF16)
        nc.sync.dma_start(out=iota_sb[:], in_=iotab[:])
        dstf_sb = const.tile([128, plan.TS], BF16)
        nc.sync.dma_start(out=dstf_sb[:], in_=dstf[:])
        ndstf_sb = const.tile([128, plan.TS], F32)
        nc.sync.dma_start(out=ndstf_sb[:], in_=ndstf[:])
        gcnt_sb = const.tile([1, plan.NG], I32)
        nc.sync.dma_start(out=gcnt_sb[:], in_=gcntt[:])
        ones_sb = const.tile([128, 1], BF16)
        nc.vector.memset(ones_sb[:], 1.0)
        idf32 = const.tile([128, 128], F32)
        make_identity(nc, idf32[:])
        idbf = const.tile([128, 128], BF16)
        make_identity(nc, idbf[:])
        acc_all = const.tile([128, n_tiles * D], F32)

        # pre-zero the G pool slots so slots skipped by -1 pads hold finite data
        for _ in range(8):
            gz = gpool.tile([128, max_slots, 128], BF16, tag="G", name="gz")
            nc.vector.memset(gz[:], 0.0)

        greg = ctx.enter_context(nc.gpsimd.register("gcnt_r"))

        # ---- phase 1: projection tables ----
        for t in range(n_tiles):
            rows = min(T, NS - t * T)
            xt = sb.tile([128, D], F32, tag="xt")
            nc.sync.dma_start(out=xt[:rows, :], in_=x[t * T:t * T + rows, :])
            xb = sb.tile([128, D], BF16, tag="xb")
            nc.vector.tensor_copy(out=xb[:rows, :], in_=xt[:rows, :])
            xTk = []
            for k in range(2):
                tp = ps_t.tile([128, 128], BF16, tag="tp")
                nc.tensor.transpose(
                    out=tp[:, :rows],
                    in_=xb[:rows, k * 128:(k + 1) * 128],
                    identity=idbf[:rows, :rows],
                )
                xT = sb.tile([128, 128], BF16, tag="xT")
                nc.vector.tensor_copy(out=xT[:, :rows], in_=tp[:, :rows])
                xTk.append(xT)
            pp = ps_w.tile([128, 512], F32, tag="wide")
            for k in range(2):
                nc.tensor.matmul(
                    out=pp[:rows, :],
                    lhsT=xTk[k][:, :rows],
                    rhs=wall_sb[k][:, :],
                    start=(k == 0),
                    stop=(k == 1),
                )
            pb = sb.tile([128, 512], BF16, tag="pb")
            nc.vector.tensor_copy(out=pb[:rows, :], in_=pp[:rows, :])
            for s in range(4):
                nc.sync.dma_start(out=proj_local[s][t * T:t * T + rows, :],
                                  in_=pb[:rows, s * 128:(s + 1) * 128])

        for s in range(4):
            nc.gpsimd.collective_compute(
                "AllGather",
                mybir.AluOpType.bypass,
                replica_groups=[list(range(n_cores))],
                ins=[proj_local[s][:]],
                outs=[tables[s][:]],
            )

        # ---- phase 2: gather + one-hot segment sum ----
        qn = 0
        for s in range(4):
            for tp_ in plan.tiles[s]:
                t = tp_.t
                if tp_.slots == 0:
                    Sp = ps_s.tile([128, 128], F32, tag="S")
                    cp = ps_c.tile([128, 1], F32, tag="cnt")
                    nc.vector.memset(Sp[:], 0.0)
                    nc.vector.memset(cp[:], 0.0)
                else:
                    G = gpool.tile([128, tp_.slots, 128], BF16, tag="G")
                    ic0 = tp_.lo.icol0
                    ict = (tp_.lo.n_idx + tp_.hi.n_idx) // 16
                    ix_t = ipool.tile([128, max_icols], I16, tag="ix")
                    nc.sync.dma_start(out=ix_t[:, :ict], in_=idx[:, ic0:ic0 + ict])
                    for seg, tbl in ((tp_.lo, tables[s][0:plan.split, :]),
                                     (tp_.hi, tables[s][plan.split:plan.N, :])):
                        for (o, n, g) in seg.chunks:
                            nc.gpsimd.load(greg, gcnt_sb[0:1, g:g + 1])
                            nc.gpsimd.dma_gather(
                                out_ap=G[:, seg.slot0 + o // T:seg.slot0 + (o + n) // T, :],
                                in_ap=tbl,
                                idxs_ap=ix_t[:, seg.icol0 - ic0 + o // 16:seg.icol0 - ic0 + (o + n) // 16],
                                num_idxs=n,
                                num_idxs_reg=greg,
                                elem_size=128,
                                elem_step=128,
                                queue_num=qn,
                            )
                            qn = (qn + 1) % 4
                    Sp = ps_s.tile([128, 128], F32, tag="S")
                    cp = ps_c.tile([128, 1], F32, tag="cnt")
                    ns = tp_.slots
                    Ob = opool.tile([128, ns, 128], BF16, tag="O")
                    dc = tp_.dcol0
                    if t % 3 == 2:
                        for j in range(ns):
                            nc.scalar.activation(
                                out=Ob[:, j, :], in_=iota_sb[:, :128],
                                func=mybir.ActivationFunctionType.Abs,
                                bias=ndstf_sb[:, dc + j:dc + j + 1], scale=1.0)
                        for j in range(ns):
                            nc.scalar.activation(
                                out=Ob[:, j, :], in_=Ob[:, j, :],
                                func=mybir.ActivationFunctionType.Relu,
                                bias=1.0, scale=-1.0)
                    else:
                        nc.vector.tensor_tensor(
                            out=Ob[:],
                            in0=dstf_sb[:, dc:dc + ns, None].to_broadcast([128, ns, 128]),
                            in1=iota_sb[:, :ns * 128],
                            op=mybir.AluOpType.is_equal,
                        )
                    for j in range(ns):
                        st, sp_ = (j == 0), (j == ns - 1)
                        nc.tensor.matmul(out=cp[:], lhsT=Ob[:, j, :], rhs=ones_sb[:],
                                         start=st, stop=sp_)
                        nc.tensor.matmul(out=Sp[:], lhsT=Ob[:, j, :], rhs=G[:, j, :],
                                         start=st, stop=sp_)
                # epilogue for (s, t)
                cm = sb.tile([128, 1], F32, tag="cm")
                nc.vector.tensor_scalar_max(out=cm[:], in0=cp[:], scalar1=1.0)
                rc = sb.tile([128, 1], F32, tag="rc")
                nc.vector.reciprocal(out=rc[:], in_=cm[:])
                tmp = sb.tile([128, 128], F32, tag="tmp")
                nc.vector.tensor_scalar_mul(out=tmp[:], in0=Sp[:], scalar1=rc[:, :1])
                nc.vector.tensor_tensor(out=tmp[:], in0=tmp[:],
                                        in1=ball_sb[:, s * 128:(s + 1) * 128],
                                        op=mybir.AluOpType.add)
                half = 0 if s < 2 else 128
                accsl = acc_all[:, t * D + half:t * D + half + 128]
                if s % 2 == 0:
                    nc.scalar.activation(out=accsl, in_=tmp[:],
                                         func=mybir.ActivationFunctionType.Relu)
                else:
                    nc.scalar.activation(out=tmp[:], in_=tmp[:],
                                         func=mybir.ActivationFunctionType.Relu)
                    nc.vector.tensor_tensor(out=accsl, in0=accsl, in1=tmp[:],
                                            op=mybir.AluOpType.add)

        # ---- phase 3: W1 matmul + residual ----
        for t in range(n_tiles):
            rows = min(T, NS - t * T)
            aTk = []
            for k in range(2):
                tp = ps_t.tile([128, 128], F32, tag="tp")
                nc.tensor.transpose(
                    out=tp[:],
                    in_=acc_all[:, t * D + k * 128:t * D + (k + 1) * 128],
                    identity=idf32[:],
                )
                aT = sb.tile([128, 128], BF16, tag="xT")
                nc.vector.tensor_copy(out=aT[:], in_=tp[:])
                aTk.append(aT)
            fp = ps_w.tile([128, D], F32, tag="wide")
            for k in range(2):
                nc.tensor.matmul(
                    out=fp[:rows, :],
                    lhsT=aTk[k][:, :rows],
                    rhs=w1_sb[k][:, :],
                    start=(k == 0),
                    stop=(k == 1),
                )
            xin = sb.tile([128, D], F32, tag="xt")
            nc.sync.dma_start(out=xin[:rows, :], in_=x[t * T:t * T + rows, :])
            ot = sb.tile([128, D], F32, tag="ot")
            nc.vector.tensor_tensor(out=ot[:rows, :], in0=fp[:rows, :],
                                    in1=b1_sb[:rows, :], op=mybir.AluOpType.add)
            nc.vector.tensor_tensor(out=ot[:rows, :], in0=ot[:rows, :],
                                    in1=xin[:rows, :], op=mybir.AluOpType.add)
            nc.sync.dma_start(out=y[t * T:t * T + rows, :], in_=ot[:rows, :])

    nc.compile()
    return nc


def reference_np(x, edge_sets, Wall, ball, W1, b1, N):
    acc = np.zeros((N, 256), np.float32)
    for s in range(4):
        src, dst = edge_sets[s][0], edge_sets[s][1]
        msg = x[src] @ Wall[:, s * 128:(s + 1) * 128]
        agg = np.zeros((N, 128), np.float32)
        np.add.at(agg, dst, msg)
        deg = np.bincount(dst, minlength=N).astype(np.float32)
        h = np.maximum(agg / np.maximum(deg, 1.0)[:, None] + ball[s * 128:(s + 1) * 128], 0.0)
        half = 0 if s < 2 else 128
        acc[:, half:half + 128] += h
    return np.maximum(acc, 0.0) @ W1 + b1 + x


def kernel(inps, fw_edges, bw_edges, W_fw, b_fw, W_bw, b_bw, W1, b1):
    """Full (unsharded) inputs in, full output out. Shards across 8 cores
    by destination node, runs the Bass kernel via run_bass_kernel_spmd."""
    from concourse.bass_utils import run_bass_kernel_spmd

    inps = np.asarray(inps)
    N = inps.shape[0]
    n_cores = 8
    Wall = np.concatenate([np.asarray(W_bw)[0], np.asarray(W_bw)[1],
                           np.asarray(W_fw)[0], np.asarray(W_fw)[1]], axis=1)
    ball = np.concatenate([np.asarray(b_bw)[0], np.asarray(b_bw)[1],
                           np.asarray(b_fw)[0], np.asarray(b_fw)[1]])
    edge_sets = [np.asarray(bw_edges)[0], np.asarray(bw_edges)[1],
                 np.asarray(fw_edges)[0], np.asarray(fw_edges)[1]]
    plan = make_plan(edge_sets, N, n_cores, split=32768)
    nc = build_nc(plan)
    in_maps = [host_inputs(plan, c, inps, Wall, ball, np.asarray(W1), np.asarray(b1))
               for c in range(n_cores)]
    res = run_bass_kernel_spmd(nc, in_maps, core_ids=list(range(n_cores)))
    out = np.concatenate([res.results[c]["y"] for c in range(n_cores)], axis=0)
    return out.astype(np.float32)

